# revision 34
# baseline (speedup 1.0000x reference)
"""Trainium2 Bass kernel for nn_AnotherDDoIGRUCell.

Math (per timestep, parallel part folded on host):
  x_tm1 = x_{t-1} @ M,  x_tm2 = x_{t-2} @ M @ M   with M = C.T @ C
  d_x   = x_t - x_tm1 ; dd_x = x_t - 2*x_tm1 + x_tm2
  r = sigmoid(x_t@Wxr + d_x@dWxr + dd_x@ddWxr + h@Whr + br)
  u = sigmoid(x_t@Wxu + d_x@dWxu + dd_x@ddWxu + h@Whu + bu)
  c = tanh   (x_t@Wxh + (r*h)@Whh + bh)
  h = u*h + (1-u)*c

The x-projections collapse to 3 effective weights per gate applied to
x_t, x_{t-1}, x_{t-2}. They are computed (together with the bias, via a
tiny indicator matmul) straight into the recurrence PSUM accumulators in
4-step groups, so no separate pre-activation buffer or copies exist.
Per-step chain: MM(r) -> sigmoid -> r*h -> MM(c) -> tanh -> 2 DVE ops;
the u-path (sigmoid, 1-u, u*h) runs on ACT/Pool off the chain.

Sharding: pure data parallel, batch 128 -> 16 rows per core x 8 cores.
"""

import sys
import numpy as np

sys.path.insert(0, "/opt/trn_rl_repo")

import concourse.bass as bass
import concourse.bacc as bacc
import concourse.tile as tile
from concourse import mybir
from concourse.masks import make_identity
from concourse.bass_utils import run_bass_kernel_spmd

B, T, IND, U = 128, 1024, 128, 256
NCORES = 8
BL = B // NCORES          # 16 batch rows per core
CHUNK = 64                # timesteps per ring/output chunk
GQ = 4                    # timesteps per PSUM accumulation group
F32 = mybir.dt.float32
BF16 = mybir.dt.bfloat16

_cache = {}


def _build_program():
    nc = bacc.Bacc()

    xc = nc.declare_dram_parameter("xc", [BL, T, IND], F32, isOutput=False)
    wa = nc.declare_dram_parameter("wa", [14, 128, 128], F32, isOutput=False)
    wh = nc.declare_dram_parameter("wh", [12, 128, 128], F32, isOutput=False)
    aux = nc.declare_dram_parameter("aux", [128, 4 * GQ * 16], F32, isOutput=False)
    out = nc.declare_dram_parameter("out", [BL, T, U], F32, isOutput=True)

    # projection term list: terms[gj] = [(wa_idx, shift), ...]
    terms = []
    widx = 0
    for gj in range(6):
        g = gj // 2
        if g < 2:
            terms.append([(widx, 0), (widx + 1, 16), (widx + 2, 32)])
            widx += 3
        else:
            terms.append([(widx, 0)])
            widx += 1
    assert widx == 14

    Sig = mybir.ActivationFunctionType.Sigmoid
    Tanh = mybir.ActivationFunctionType.Tanh
    Add = mybir.AluOpType.add
    Mult = mybir.AluOpType.mult

    with tile.TileContext(nc) as tc:
        with (
            tc.tile_pool(name="singles", bufs=1) as singles,
            tc.tile_pool(name="xT", bufs=1) as xT_pool,
            tc.tile_pool(name="xstage", bufs=4) as xstage,
            tc.tile_pool(name="tpsum", bufs=2, space="PSUM") as tpsum,
            tc.tile_pool(name="gps4", bufs=2, space="PSUM") as gps4,
            tc.tile_pool(name="gps4c", bufs=2, space="PSUM") as gps4c,
            tc.tile_pool(name="ring", bufs=2) as ringp,
            tc.tile_pool(name="small", bufs=3) as small,
        ):
            # --- resident tensors ---
            wa_f32 = singles.tile([128, 14, 128], F32)
            nc.sync.dma_start(out=wa_f32, in_=wa.rearrange("w p f -> p w f"))
            wa_sb = singles.tile([128, 14, 128], BF16)
            nc.vector.tensor_copy(wa_sb, wa_f32)
            wh_f32 = singles.tile([128, 12, 128], F32)
            nc.sync.dma_start(out=wh_f32, in_=wh.rearrange("w p f -> p w f"))
            wh_sb = singles.tile([128, 12, 128], BF16)
            nc.vector.tensor_copy(wh_sb, wh_f32)
            aux_sb = singles.tile([128, 4 * GQ * 16], F32)
            nc.sync.dma_start(out=aux_sb, in_=aux[:])
            ind_ru = singles.tile([4, 4 * GQ * 16], BF16)
            nc.vector.tensor_copy(ind_ru, aux_sb[0:4, :])
            ind_c = singles.tile([2, 2 * GQ * 16], BF16)
            nc.vector.tensor_copy(ind_c, aux_sb[32:34, 0:2 * GQ * 16])
            bias_ru = singles.tile([4, 128], BF16)
            nc.vector.tensor_copy(bias_ru, aux_sb[64:68, 0:128])
            bias_c = singles.tile([2, 128], BF16)
            nc.vector.tensor_copy(bias_c, aux_sb[96:98, 0:128])
            idt = singles.tile([128, 128], F32)
            make_identity(nc, idt)
            idt_bf = singles.tile([128, 128], BF16)
            nc.vector.tensor_copy(idt_bf, idt)
            h0 = singles.tile([128, 32], BF16)
            nc.vector.memset(h0, 0.0)

            # --- transpose x into xT[p=in_dim, col=t*16+b] (bf16) ---
            xT = xT_pool.tile([128, T * BL], BF16)
            xT_v = xT.rearrange("p (t b) -> p t b", b=BL)
            dma_engines = [nc.sync, nc.scalar]
            for b in range(BL):
                xs = xstage.tile([128, T // 128, 128], F32, tag="xs")
                dma_engines[b % 2].dma_start(
                    out=xs,
                    in_=xc[b].rearrange("(tt p) d -> p tt d", p=128))
                for tt in range(T // 128):
                    ps = tpsum.tile([128, 128], F32, tag="tp")
                    nc.tensor.transpose(ps, xs[:, tt, :], idt)
                    nc.vector.tensor_copy(
                        xT_v[:, tt * 128:(tt + 1) * 128, b], ps)

            # --- recurrence ---
            # group PSUM layouts:
            #   prug [128, 256]: col = gi*64 + tq*16 + b   (gi: r0 r1 u0 u1)
            #   pcg  [128, 128]: col = j*64 + tq*16 + b    (j: c0 c1)
            def emit_group_proj(prug, pcg, tg0):
                """projection + bias matmuls for steps [tg0, tg0+GQ)"""
                gc0 = tg0 * 16          # xT col of first step
                ncols = GQ * 16
                # bias via indicator matmuls: full-width, the single
                # start=True initializer of each psum tile
                nc.tensor.matmul(prug, lhsT=bias_ru, rhs=ind_ru,
                                 start=True, stop=False,
                                 skip_group_check=True)
                nc.tensor.matmul(pcg, lhsT=bias_c, rhs=ind_c,
                                 start=True, stop=False,
                                 skip_group_check=True)
                for gi in range(4):
                    g, j = gi // 2, gi % 2
                    for wi, shift in terms[g * 2 + j]:
                        o = max(0, shift - gc0)   # skip cols before t=0
                        nc.tensor.matmul(
                            prug[:, gi * ncols + o:(gi + 1) * ncols],
                            lhsT=wa_sb[:, wi, :],
                            rhs=xT[:, gc0 + o - shift:gc0 + ncols - shift],
                            start=False, stop=False, skip_group_check=True)
                for j in range(2):
                    wi0 = terms[4 + j][0][0]
                    nc.tensor.matmul(
                        pcg[:, j * ncols:(j + 1) * ncols],
                        lhsT=wa_sb[:, wi0, :], rhs=xT[:, gc0:gc0 + ncols],
                        start=False, stop=False, skip_group_check=True)

            hT = h0
            prug = pcg = None
            for chunk in range(T // CHUNK):
                t0 = chunk * CHUNK
                ring = ringp.tile([128, CHUNK * 32], BF16, tag="ring")
                for tl in range(CHUNK):
                    tq = tl % GQ
                    if tq == 0:
                        prug = gps4.tile([128, 4 * GQ * 16], F32, tag="g4")
                        pcg = gps4c.tile([128, 2 * GQ * 16], F32, tag="g4c")
                        emit_group_proj(prug, pcg, t0 + tl)
                    q0 = tq * 16
                    # r/u h-matmuls accumulate into this step's psum cols
                    for gi in range(4):          # r0 r1 u0 u1
                        for k in range(2):
                            nc.tensor.matmul(
                                prug[:, gi * GQ * 16 + q0:gi * GQ * 16 + q0 + 16],
                                lhsT=wh_sb[:, (gi // 2) * 4
                                           + (gi % 2) * 2 + k, :],
                                rhs=hT[:, k * 16:(k + 1) * 16],
                                start=False, stop=(k == 1),
                                skip_group_check=True)
                    pr_v = prug.rearrange("p (g x) -> p g x", g=4)
                    r_sb = small.tile([128, 32], F32, tag="r")
                    nc.scalar.activation(
                        r_sb.rearrange("p (g x) -> p g x", g=2),
                        pr_v[:, 0:2, q0:q0 + 16], Sig)
                    rh = small.tile([128, 32], BF16, tag="rh")
                    nc.vector.tensor_mul(rh, r_sb, hT)
                    # candidate
                    for j in range(2):
                        for k in range(2):
                            nc.tensor.matmul(
                                pcg[:, j * GQ * 16 + q0:j * GQ * 16 + q0 + 16],
                                lhsT=wh_sb[:, 8 + j * 2 + k, :],
                                rhs=rh[:, k * 16:(k + 1) * 16],
                                start=False, stop=(k == 1),
                                skip_group_check=True)
                    # u path (off the critical chain)
                    u_sb = small.tile([128, 32], F32, tag="u")
                    nc.scalar.activation(
                        u_sb.rearrange("p (g x) -> p g x", g=2),
                        pr_v[:, 2:4, q0:q0 + 16], Sig)
                    uc = small.tile([128, 32], F32, tag="uc")
                    nc.gpsimd.tensor_scalar(uc, u_sb, -1.0, 1.0, Mult, Add)
                    t1 = small.tile([128, 32], F32, tag="t1")
                    nc.gpsimd.tensor_mul(t1, u_sb, hT)
                    pc_v = pcg.rearrange("p (g x) -> p g x", g=2)
                    c_sb = small.tile([128, 32], F32, tag="c")
                    nc.scalar.activation(
                        c_sb.rearrange("p (g x) -> p g x", g=2),
                        pc_v[:, :, q0:q0 + 16], Tanh)
                    m2 = small.tile([128, 32], F32, tag="m2")
                    nc.vector.tensor_mul(m2, uc, c_sb)
                    hnew = ring[:, tl * 32:(tl + 1) * 32]
                    nc.vector.tensor_add(hnew, m2, t1)
                    hT = hnew

                    # transpose each finished 4-step group back to
                    # batch-major [(t,c,b), u] and DMA out
                    if (tl + 1) % 4 == 0:
                        tg = (tl + 1) // 4 - 1
                        tp = tpsum.tile([128, 128], BF16, tag="tpo")
                        nc.tensor.transpose(
                            tp, ring[:, tg * 128:(tg + 1) * 128], idt_bf)
                        ob = small.tile([128, 128], F32, tag="ob")
                        if tg % 2 == 0:
                            nc.scalar.copy(ob, tp)
                        else:
                            nc.vector.tensor_copy(ob, tp)
                        oc = out[:, t0 + tg * 4:t0 + (tg + 1) * 4, :]
                        nc.sync.dma_start(
                            out=oc.rearrange("b t (c p) -> (t c) b p", c=2),
                            in_=ob)
    nc.compile()
    return nc


def _fold_weights(input_weight, hidden_weight, bias, constant):
    iw = np.asarray(input_weight, np.float64)
    hw = np.asarray(hidden_weight, np.float64)
    bs = np.asarray(bias, np.float64)
    C = np.asarray(constant, np.float64)
    Wxr, Wxu, Wxh, dWxr, dWxu, ddWxr, ddWxu = [
        iw[:, i * U:(i + 1) * U] for i in range(7)]
    M = C.T @ C
    M2 = M @ M
    eff = {
        0: (Wxr + dWxr + ddWxr, -M @ (dWxr + 2 * ddWxr), M2 @ ddWxr),
        1: (Wxu + dWxu + ddWxu, -M @ (dWxu + 2 * ddWxu), M2 @ ddWxu),
        2: (Wxh,),
    }
    WA = np.zeros((14, 128, 128), np.float32)
    wi = 0
    for gj in range(6):
        g, j = gj // 2, gj % 2
        for Wt in eff[g]:
            WA[wi] = Wt[:, j * 128:(j + 1) * 128].astype(np.float32)
            wi += 1
    assert wi == 14
    WH = np.zeros((12, 128, 128), np.float32)
    for g in range(3):
        Whg = hw[:, g * U:(g + 1) * U]
        for j in range(2):
            for k in range(2):
                WH[g * 4 + j * 2 + k] = Whg[
                    k * 128:(k + 1) * 128, j * 128:(j + 1) * 128
                ].astype(np.float32)
    # aux: indicator + bias rows (32-aligned) for in-psum bias matmuls
    blk = GQ * 16
    AUX = np.zeros((128, 4 * blk), np.float32)
    for k in range(4):                       # ind_ru[k, c] = (c//blk == k)
        AUX[k, k * blk:(k + 1) * blk] = 1.0
    for k in range(2):                       # ind_c[k, c] = (c//blk == k)
        AUX[32 + k, k * blk:(k + 1) * blk] = 1.0
    for gi in range(4):                      # bias r0 r1 u0 u1
        g, j = gi // 2, gi % 2
        AUX[64 + gi, 0:128] = bs[g * U + j * 128:g * U + (j + 1) * 128]
    for j in range(2):                       # bias c0 c1
        AUX[96 + j, 0:128] = bs[2 * U + j * 128:2 * U + (j + 1) * 128]
    return WA, WH, AUX


def prepare(x, input_weight, hidden_weight, bias, constant):
    x = np.ascontiguousarray(np.asarray(x, np.float32))
    WA, WH, AUX = _fold_weights(input_weight, hidden_weight, bias, constant)
    if "nc" not in _cache:
        _cache["nc"] = _build_program()
    in_maps = [
        {"xc": np.ascontiguousarray(x[i * BL:(i + 1) * BL]),
         "wa": WA, "wh": WH, "aux": AUX}
        for i in range(NCORES)
    ]
    return _cache["nc"], in_maps


def kernel(x, input_weight, hidden_weight, bias, constant):
    nc, in_maps = prepare(x, input_weight, hidden_weight, bias, constant)
    res = run_bass_kernel_spmd(nc, in_maps, list(range(NCORES)))
    outs = [res.results[i]["out"] for i in range(NCORES)]
    return np.concatenate(outs, axis=0)


if __name__ == "__main__":
    rng = np.random.default_rng(0)
    x = rng.standard_normal((B, T, IND), dtype=np.float32)
    iw = (rng.standard_normal((IND, 7 * U)) * 0.05).astype(np.float32)
    hw = (rng.standard_normal((U, 3 * U)) * 0.05).astype(np.float32)
    bs = np.zeros(3 * U, np.float32)
    C = np.concatenate([np.eye(IND, dtype=np.float32),
                        np.zeros((U - IND, IND), np.float32)], 0)
    y = kernel(x, iw, hw, bs, C)
    print("out", y.shape, y.dtype, float(np.abs(y).mean()))


# revision 36
# speedup vs baseline: 1.0071x; 1.0071x over previous
"""Trainium2 Bass kernel for nn_AnotherDDoIGRUCell.

Math (per timestep, parallel part folded on host):
  x_tm1 = x_{t-1} @ M,  x_tm2 = x_{t-2} @ M @ M   with M = C.T @ C
  d_x   = x_t - x_tm1 ; dd_x = x_t - 2*x_tm1 + x_tm2
  r = sigmoid(x_t@Wxr + d_x@dWxr + dd_x@ddWxr + h@Whr + br)
  u = sigmoid(x_t@Wxu + d_x@dWxu + dd_x@ddWxu + h@Whu + bu)
  c = tanh   (x_t@Wxh + (r*h)@Whh + bh)
  h = u*h + (1-u)*c

The x-projections collapse to 3 effective weights per gate applied to
x_t, x_{t-1}, x_{t-2}. They are computed (together with the bias, via a
tiny indicator matmul) straight into the recurrence PSUM accumulators in
4-step groups, so no separate pre-activation buffer or copies exist.
Per-step chain: MM(r) -> sigmoid -> r*h -> MM(c) -> tanh -> 2 DVE ops;
the u-path (sigmoid, 1-u, u*h) runs on ACT/Pool off the chain.

Sharding: pure data parallel, batch 128 -> 16 rows per core x 8 cores.
"""

import sys
import numpy as np

sys.path.insert(0, "/opt/trn_rl_repo")

import concourse.bass as bass
import concourse.bacc as bacc
import concourse.tile as tile
from concourse import mybir
from concourse.masks import make_identity
from concourse.bass_utils import run_bass_kernel_spmd

B, T, IND, U = 128, 1024, 128, 256
NCORES = 8
BL = B // NCORES          # 16 batch rows per core
CHUNK = 64                # timesteps per ring/output chunk
GQ = 4                    # timesteps per PSUM accumulation group
F32 = mybir.dt.float32
BF16 = mybir.dt.bfloat16

_cache = {}


def _build_program():
    nc = bacc.Bacc()

    xc = nc.declare_dram_parameter("xc", [BL, T, IND], F32, isOutput=False)
    wa = nc.declare_dram_parameter("wa", [14, 128, 128], F32, isOutput=False)
    wh = nc.declare_dram_parameter("wh", [12, 128, 128], F32, isOutput=False)
    aux = nc.declare_dram_parameter("aux", [128, 4 * GQ * 16], F32, isOutput=False)
    out = nc.declare_dram_parameter("out", [BL, T, U], F32, isOutput=True)

    # projection term list: terms[gj] = [(wa_idx, shift), ...]
    terms = []
    widx = 0
    for gj in range(6):
        g = gj // 2
        if g < 2:
            terms.append([(widx, 0), (widx + 1, 16), (widx + 2, 32)])
            widx += 3
        else:
            terms.append([(widx, 0)])
            widx += 1
    assert widx == 14

    Sig = mybir.ActivationFunctionType.Sigmoid
    Tanh = mybir.ActivationFunctionType.Tanh
    Add = mybir.AluOpType.add
    Mult = mybir.AluOpType.mult

    with tile.TileContext(nc) as tc:
        with (
            tc.tile_pool(name="singles", bufs=1) as singles,
            tc.tile_pool(name="xT", bufs=1) as xT_pool,
            tc.tile_pool(name="xstage", bufs=4) as xstage,
            tc.tile_pool(name="tpsum", bufs=2, space="PSUM") as tpsum,
            tc.tile_pool(name="gps4", bufs=2, space="PSUM") as gps4,
            tc.tile_pool(name="gps4c", bufs=2, space="PSUM") as gps4c,
            tc.tile_pool(name="ring", bufs=2) as ringp,
            tc.tile_pool(name="small", bufs=3) as small,
        ):
            # --- resident tensors ---
            wa_f32 = singles.tile([128, 14, 128], F32)
            nc.sync.dma_start(out=wa_f32, in_=wa.rearrange("w p f -> p w f"))
            wa_sb = singles.tile([128, 14, 128], BF16)
            nc.vector.tensor_copy(wa_sb, wa_f32)
            wh_f32 = singles.tile([128, 12, 128], F32)
            nc.sync.dma_start(out=wh_f32, in_=wh.rearrange("w p f -> p w f"))
            wh_sb = singles.tile([128, 12, 128], BF16)
            nc.vector.tensor_copy(wh_sb, wh_f32)
            aux_sb = singles.tile([128, 4 * GQ * 16], F32)
            nc.sync.dma_start(out=aux_sb, in_=aux[:])
            ind_ru = singles.tile([4, 4 * GQ * 16], BF16)
            nc.vector.tensor_copy(ind_ru, aux_sb[0:4, :])
            ind_c = singles.tile([2, 2 * GQ * 16], BF16)
            nc.vector.tensor_copy(ind_c, aux_sb[32:34, 0:2 * GQ * 16])
            bias_ru = singles.tile([4, 128], BF16)
            nc.vector.tensor_copy(bias_ru, aux_sb[64:68, 0:128])
            bias_c = singles.tile([2, 128], BF16)
            nc.vector.tensor_copy(bias_c, aux_sb[96:98, 0:128])
            idt = singles.tile([128, 128], F32)
            make_identity(nc, idt)
            idt_bf = singles.tile([128, 128], BF16)
            nc.vector.tensor_copy(idt_bf, idt)
            h0 = singles.tile([128, 32], BF16)
            nc.vector.memset(h0, 0.0)

            # --- transpose x into xT[p=in_dim, col=t*16+b] (bf16) ---
            # Only tt=0 (t<128, covering chunks 0-1) is transposed up
            # front; the remaining tt blocks stream into the chunk loop
            # so the recurrence starts ~10us in instead of ~40us.
            xT = xT_pool.tile([128, T * BL], BF16)
            xT_v = xT.rearrange("p (t b) -> p t b", b=BL)
            dma_engines = [nc.sync, nc.scalar]

            def emit_xpose(tt, b):
                xs = xstage.tile([128, 128], F32, tag="xs")
                dma_engines[(tt * BL + b) % 2].dma_start(
                    out=xs, in_=xc[b, tt * 128:(tt + 1) * 128, :])
                ps = tpsum.tile([128, 128], F32, tag="tp")
                nc.tensor.transpose(ps, xs, idt)
                nc.vector.tensor_copy(
                    xT_v[:, tt * 128:(tt + 1) * 128, b], ps)

            for b in range(BL):
                emit_xpose(0, b)

            # --- recurrence ---
            # group PSUM layouts:
            #   prug [128, 256]: col = gi*64 + tq*16 + b   (gi: r0 r1 u0 u1)
            #   pcg  [128, 128]: col = j*64 + tq*16 + b    (j: c0 c1)
            def emit_group_proj(prug, pcg, tg0):
                """projection + bias matmuls for steps [tg0, tg0+GQ)"""
                gc0 = tg0 * 16          # xT col of first step
                ncols = GQ * 16
                # bias via indicator matmuls: full-width, the single
                # start=True initializer of each psum tile
                nc.tensor.matmul(prug, lhsT=bias_ru, rhs=ind_ru,
                                 start=True, stop=False,
                                 skip_group_check=True)
                nc.tensor.matmul(pcg, lhsT=bias_c, rhs=ind_c,
                                 start=True, stop=False,
                                 skip_group_check=True)
                for gi in range(4):
                    g, j = gi // 2, gi % 2
                    for wi, shift in terms[g * 2 + j]:
                        o = max(0, shift - gc0)   # skip cols before t=0
                        nc.tensor.matmul(
                            prug[:, gi * ncols + o:(gi + 1) * ncols],
                            lhsT=wa_sb[:, wi, :],
                            rhs=xT[:, gc0 + o - shift:gc0 + ncols - shift],
                            start=False, stop=False, skip_group_check=True)
                for j in range(2):
                    wi0 = terms[4 + j][0][0]
                    nc.tensor.matmul(
                        pcg[:, j * ncols:(j + 1) * ncols],
                        lhsT=wa_sb[:, wi0, :], rhs=xT[:, gc0:gc0 + ncols],
                        start=False, stop=False, skip_group_check=True)

            hT = h0
            prug = pcg = None
            for chunk in range(T // CHUNK):
                t0 = chunk * CHUNK
                ring = ringp.tile([128, CHUNK * 32], BF16, tag="ring")
                for tl in range(CHUNK):
                    tq = tl % GQ
                    if tq == 0:
                        prug = gps4.tile([128, 4 * GQ * 16], F32, tag="g4")
                        pcg = gps4c.tile([128, 2 * GQ * 16], F32, tag="g4c")
                        emit_group_proj(prug, pcg, t0 + tl)
                        # stream one future x-transpose per group: during
                        # even chunk c, transpose block tt = c//2 + 1
                        if chunk % 2 == 0 and chunk // 2 + 1 < T // 128:
                            emit_xpose(chunk // 2 + 1, tl // GQ)
                    q0 = tq * 16
                    # r/u h-matmuls accumulate into this step's psum cols
                    for gi in range(4):          # r0 r1 u0 u1
                        for k in range(2):
                            nc.tensor.matmul(
                                prug[:, gi * GQ * 16 + q0:gi * GQ * 16 + q0 + 16],
                                lhsT=wh_sb[:, (gi // 2) * 4
                                           + (gi % 2) * 2 + k, :],
                                rhs=hT[:, k * 16:(k + 1) * 16],
                                start=False, stop=(k == 1),
                                skip_group_check=True)
                    pr_v = prug.rearrange("p (g x) -> p g x", g=4)
                    r_sb = small.tile([128, 32], F32, tag="r")
                    nc.scalar.activation(
                        r_sb.rearrange("p (g x) -> p g x", g=2),
                        pr_v[:, 0:2, q0:q0 + 16], Sig)
                    rh = small.tile([128, 32], BF16, tag="rh")
                    nc.vector.tensor_mul(rh, r_sb, hT)
                    # candidate
                    for j in range(2):
                        for k in range(2):
                            nc.tensor.matmul(
                                pcg[:, j * GQ * 16 + q0:j * GQ * 16 + q0 + 16],
                                lhsT=wh_sb[:, 8 + j * 2 + k, :],
                                rhs=rh[:, k * 16:(k + 1) * 16],
                                start=False, stop=(k == 1),
                                skip_group_check=True)
                    # u path (off the critical chain)
                    u_sb = small.tile([128, 32], F32, tag="u")
                    nc.scalar.activation(
                        u_sb.rearrange("p (g x) -> p g x", g=2),
                        pr_v[:, 2:4, q0:q0 + 16], Sig)
                    uc = small.tile([128, 32], F32, tag="uc")
                    nc.gpsimd.tensor_scalar(uc, u_sb, -1.0, 1.0, Mult, Add)
                    t1 = small.tile([128, 32], F32, tag="t1")
                    nc.gpsimd.tensor_mul(t1, u_sb, hT)
                    pc_v = pcg.rearrange("p (g x) -> p g x", g=2)
                    c_sb = small.tile([128, 32], F32, tag="c")
                    nc.scalar.activation(
                        c_sb.rearrange("p (g x) -> p g x", g=2),
                        pc_v[:, :, q0:q0 + 16], Tanh)
                    m2 = small.tile([128, 32], F32, tag="m2")
                    nc.vector.tensor_mul(m2, uc, c_sb)
                    hnew = ring[:, tl * 32:(tl + 1) * 32]
                    nc.vector.tensor_add(hnew, m2, t1)
                    hT = hnew

                    # transpose each finished 4-step group back to
                    # batch-major [(t,c,b), u] and DMA out
                    if (tl + 1) % 4 == 0:
                        tg = (tl + 1) // 4 - 1
                        tp = tpsum.tile([128, 128], BF16, tag="tpo")
                        nc.tensor.transpose(
                            tp, ring[:, tg * 128:(tg + 1) * 128], idt_bf)
                        ob = small.tile([128, 128], F32, tag="ob")
                        if tg % 2 == 0:
                            nc.scalar.copy(ob, tp)
                        else:
                            nc.vector.tensor_copy(ob, tp)
                        oc = out[:, t0 + tg * 4:t0 + (tg + 1) * 4, :]
                        nc.sync.dma_start(
                            out=oc.rearrange("b t (c p) -> (t c) b p", c=2),
                            in_=ob)
    nc.compile()
    return nc


def _fold_weights(input_weight, hidden_weight, bias, constant):
    iw = np.asarray(input_weight, np.float64)
    hw = np.asarray(hidden_weight, np.float64)
    bs = np.asarray(bias, np.float64)
    C = np.asarray(constant, np.float64)
    Wxr, Wxu, Wxh, dWxr, dWxu, ddWxr, ddWxu = [
        iw[:, i * U:(i + 1) * U] for i in range(7)]
    M = C.T @ C
    M2 = M @ M
    eff = {
        0: (Wxr + dWxr + ddWxr, -M @ (dWxr + 2 * ddWxr), M2 @ ddWxr),
        1: (Wxu + dWxu + ddWxu, -M @ (dWxu + 2 * ddWxu), M2 @ ddWxu),
        2: (Wxh,),
    }
    WA = np.zeros((14, 128, 128), np.float32)
    wi = 0
    for gj in range(6):
        g, j = gj // 2, gj % 2
        for Wt in eff[g]:
            WA[wi] = Wt[:, j * 128:(j + 1) * 128].astype(np.float32)
            wi += 1
    assert wi == 14
    WH = np.zeros((12, 128, 128), np.float32)
    for g in range(3):
        Whg = hw[:, g * U:(g + 1) * U]
        for j in range(2):
            for k in range(2):
                WH[g * 4 + j * 2 + k] = Whg[
                    k * 128:(k + 1) * 128, j * 128:(j + 1) * 128
                ].astype(np.float32)
    # aux: indicator + bias rows (32-aligned) for in-psum bias matmuls
    blk = GQ * 16
    AUX = np.zeros((128, 4 * blk), np.float32)
    for k in range(4):                       # ind_ru[k, c] = (c//blk == k)
        AUX[k, k * blk:(k + 1) * blk] = 1.0
    for k in range(2):                       # ind_c[k, c] = (c//blk == k)
        AUX[32 + k, k * blk:(k + 1) * blk] = 1.0
    for gi in range(4):                      # bias r0 r1 u0 u1
        g, j = gi // 2, gi % 2
        AUX[64 + gi, 0:128] = bs[g * U + j * 128:g * U + (j + 1) * 128]
    for j in range(2):                       # bias c0 c1
        AUX[96 + j, 0:128] = bs[2 * U + j * 128:2 * U + (j + 1) * 128]
    return WA, WH, AUX


def prepare(x, input_weight, hidden_weight, bias, constant):
    x = np.ascontiguousarray(np.asarray(x, np.float32))
    WA, WH, AUX = _fold_weights(input_weight, hidden_weight, bias, constant)
    if "nc" not in _cache:
        _cache["nc"] = _build_program()
    in_maps = [
        {"xc": np.ascontiguousarray(x[i * BL:(i + 1) * BL]),
         "wa": WA, "wh": WH, "aux": AUX}
        for i in range(NCORES)
    ]
    return _cache["nc"], in_maps


def kernel(x, input_weight, hidden_weight, bias, constant):
    nc, in_maps = prepare(x, input_weight, hidden_weight, bias, constant)
    res = run_bass_kernel_spmd(nc, in_maps, list(range(NCORES)))
    outs = [res.results[i]["out"] for i in range(NCORES)]
    return np.concatenate(outs, axis=0)


if __name__ == "__main__":
    rng = np.random.default_rng(0)
    x = rng.standard_normal((B, T, IND), dtype=np.float32)
    iw = (rng.standard_normal((IND, 7 * U)) * 0.05).astype(np.float32)
    hw = (rng.standard_normal((U, 3 * U)) * 0.05).astype(np.float32)
    bs = np.zeros(3 * U, np.float32)
    C = np.concatenate([np.eye(IND, dtype=np.float32),
                        np.zeros((U - IND, IND), np.float32)], 0)
    y = kernel(x, iw, hw, bs, C)
    print("out", y.shape, y.dtype, float(np.abs(y).mean()))
